# revision 31
# baseline (speedup 1.0000x reference)
"""Trainium2 Bass kernel for nn_MihGNNEmbeddingTest3 (gnn_message_passing).

Reference math:
    H = mlp(A_s @ emb)          (mlp = 3 linear layers, no activations)
    out[e] = relu(|<H[src_e], H[dst_e]>| / (||H[src_e]|| ||H[dst_e]||))

Since the mlp is affine, fold it:  H = A_s @ (emb @ W_eff^T) + b_eff
(E2 = emb @ W_eff^T precomputed on host).  cos is scale-invariant, so E2
can be globally rescaled to fit fp8 range.

Layout: edge-pre-permuted, collective-free.  Each core computes
H^T columns for exactly the 2048 endpoint nodes of its own 1024 edges
(host gathers the needed A_s rows per core), via fp8-e4m3 DoubleRow
matmuls (K=256 per pass, moving free dim 512).  A_s is shipped as the
residual A-0.5 (quantization error scales with the residual, not the
value; the mean's contribution 0.5*colsum(E2) folds into the bias).
Bias lands via the per-partition bias of the ACT copy that stages
H^T out of PSUM.  dot/||h||^2 reduce over d (the partition dim) with
data-stationary matmuls against a ones column; the final
|dot|*rsqrt(ns*nd) runs on [128, 8] tiles at full lane parallelism.

Columns per core are grouped in 4 blocks of 512 = [src 256 | dst 256]
so each block's dot/norm math reads one PSUM tile; blocks are split in
2 phases of 2 so phase-0 reductions overlap phase-1 matmuls.
"""

import os
import sys

import numpy as np

try:
    import concourse.bass  # noqa: F401
except ImportError:  # pragma: no cover - grading env should have PYTHONPATH set
    for p in ("/opt/trn_rl_repo", "/root/.axon_site/_ro/trn_rl_repo"):
        if os.path.isdir(p) and p not in sys.path:
            sys.path.insert(0, p)

import ml_dtypes

N, D, B = 8192, 256, 8192
N_CORES = 8
EPC = B // N_CORES    # edges per core
COLS = 2 * EPC        # H^T columns per core (src+dst)
KT2 = N // 256        # DoubleRow k-steps (256 contraction each)
JT = EPC // 128       # edge blocks per core
NPH = 2               # phases (2 column-blocks each)
E2_SCALE_TARGET = 200.0

_CACHE = {}
LAST_RESULTS = None  # BassKernelResults of the most recent run (for test.py)


def _build():
    import concourse.bacc as bacc
    import concourse.bass as bass  # noqa: F401
    import concourse.mybir as mybir
    import concourse.tile as tile

    fp32 = mybir.dt.float32
    bf16 = mybir.dt.bfloat16
    fp8 = mybir.dt.float8e4
    DR = mybir.MatmulPerfMode.DoubleRow

    nc = bacc.Bacc(num_devices=N_CORES)
    # a8[p, cb, t, ko, col] = Rq[node(cb*512+col), t*256 + ko*128 + p]
    a8 = nc.declare_dram_parameter(
        "a8", [128, 4, KT2, 2, 512], fp8, isOutput=False
    )
    # e28[p, t, ko, d] = E2q[t*256 + ko*128 + p, d]
    e28 = nc.declare_dram_parameter("e28", [128, KT2, 2, D], fp8, isOutput=False)
    # bias pre-broadcast along the free dim so a single DVE tensor_tensor
    # does PSUM->SBUF staging + bias add (keeps ACT free for the final sqrt)
    biasb = nc.declare_dram_parameter("biasb", [128, 2, D], fp32, isOutput=False)
    # raw dot|ns|nd sums; the final |dot|/sqrt(ns*nd) is host-side
    out = nc.declare_dram_parameter("out", [128, 24], fp32, isOutput=True)

    AB01 = [0, 1, 3, 5, 8, 11, 14, 18, 22, 27, 32]
    AB23 = [0, 2, 4, 8, 12, 16, 20, 24, 28, 32]
    E_BOUNDS = [0, 2, 6, 10, 14, 18, 23, 28, 32]

    with tile.TileContext(nc) as tc:
        with (
            tc.tile_pool(name="ap", bufs=1) as apool,
            tc.tile_pool(name="ep", bufs=1) as epool,
            tc.tile_pool(name="psum", bufs=8, space="PSUM") as psum,
            tc.tile_pool(name="stage", bufs=12) as stage,
            tc.tile_pool(name="const", bufs=1) as constp,
        ):
            a_t = [[None] * KT2 for _ in range(4)]
            e_t = [None] * KT2
            issue_eng = [nc.sync, nc.scalar]
            issue_i = [0]

            def _dma(o, i):
                issue_eng[issue_i[0] % 2].dma_start(out=o, in_=i)
                issue_i[0] += 1

            # bias first (scalar ring so it doesn't delay the first a chunk):
            # a late bias at the back of the DMA queue once stalled the whole
            # reduce (and the 9th PSUM buffer) behind 18MB
            bias_sb = constp.tile([128, 2, D], fp32)
            nc.scalar.dma_start(out=bias_sb[:], in_=biasb[:])

            def load_a(cb, bounds, ci):
                lo, hi = bounds[ci], bounds[ci + 1]
                ac = apool.tile(
                    [128, hi - lo, 2, 512], fp8,
                    name=f"ac_{cb}_{ci}", tag=f"ac{cb}_{ci}",
                )
                _dma(ac[:], a8[:, cb, lo:hi, :, :])
                for t in range(lo, hi):
                    a_t[cb][t] = ac[:, t - lo, :, :]

            def load_e(ci):
                lo, hi = E_BOUNDS[ci], E_BOUNDS[ci + 1]
                ec = epool.tile(
                    [128, hi - lo, 2, D], fp8, name=f"ec_{ci}", tag=f"ec{ci}"
                )
                _dma(ec[:], e28[:, lo:hi, :, :])
                for t in range(lo, hi):
                    e_t[t] = ec[:, t - lo, :, :]

            for ci in range(10):
                if ci < 8:
                    load_e(ci)
                load_a(0, AB01, ci)
                load_a(1, AB01, ci)
                load_a(2, AB01, ci)
            for ci in range(9):
                load_a(3, AB23, ci)

            ones1 = constp.tile([128, 1], bf16)
            nc.vector.memset(ones1[:], 1.0)
            junkw = constp.tile([128, 512], bf16)
            nc.vector.memset(junkw[:], 1.0)
            # preload ACT square+identity tables while DMAs stream (else
            # their 1.3us table loads land on the critical tail)
            junk = constp.tile([128, 1], fp32)
            nc.vector.memset(junk[:], 1.0)
            junk2 = constp.tile([128, 1], fp32)
            nc.scalar.square(junk2[:], junk[:])
            junk4 = constp.tile([128, 1], fp32)
            nc.scalar.activation(
                junk4[:], junk[:],
                mybir.ActivationFunctionType.Identity, bias=0.0,
            )

            # main matmuls + per-block reductions
            red = None  # [128, 24] used: dot j | ns 8+j | nd 16+j
            ps_t = [[None, None] for _ in range(4)]  # [cb][dtile]

            def reduce_cb(cb):
                nonlocal red
                if red is None:
                    # same tag as ps: cycles within the 8-buffer pool (the
                    # 9th allocation reuses a released phase-0 bank)
                    red = psum.tile([128, 512], fp32, name="red", tag="ps")
                tiles = []
                for dt in range(2):
                    ps = ps_t[cb][dt]
                    hs_sb = stage.tile(
                        [128, 256], fp32, name=f"hs_{cb}_{dt}", tag="hs", bufs=8
                    )
                    nc.vector.tensor_tensor(
                        out=hs_sb[:], in0=ps[:, 0:256], in1=bias_sb[:, dt, :],
                        op=mybir.AluOpType.add,
                    )
                    hd_sb = stage.tile(
                        [128, 256], fp32, name=f"hd_{cb}_{dt}", tag="hd", bufs=8
                    )
                    nc.vector.tensor_tensor(
                        out=hd_sb[:], in0=ps[:, 256:512], in1=bias_sb[:, dt, :],
                        op=mybir.AluOpType.add,
                    )
                    P = stage.tile([128, 256], bf16, name=f"P_{cb}_{dt}", tag="P")
                    nc.vector.tensor_tensor(
                        out=P[:], in0=hs_sb[:], in1=hd_sb[:],
                        op=mybir.AluOpType.mult,
                    )
                    S2 = stage.tile([128, 256], bf16, name=f"S2_{cb}_{dt}", tag="S2")
                    nc.scalar.square(S2[:], hs_sb[:])
                    D2 = stage.tile([128, 256], bf16, name=f"D2_{cb}_{dt}", tag="D2")
                    nc.scalar.square(D2[:], hd_sb[:])
                    tiles.append((P, S2, D2))
                # combine the two d-tile partials on DVE so each (chunk,
                # quantity) takes a single reduce matmul (halves PE LDW cost)
                comb = []
                for q in range(3):
                    cq = stage.tile(
                        [128, 256], bf16, name=f"cq_{cb}_{q}", tag=f"cq{q}",
                        bufs=3,
                    )
                    nc.vector.tensor_tensor(
                        out=cq[:], in0=tiles[0][q][:], in1=tiles[1][q][:],
                        op=mybir.AluOpType.add,
                    )
                    comb.append(cq)
                for c in range(2):
                    j = cb * 2 + c
                    for q in range(3):
                        nc.tensor.matmul(
                            out=red[:, 8 * q + j:8 * q + j + 1],
                            lhsT=comb[q][:, c * 128:(c + 1) * 128],
                            rhs=ones1[:],
                            start=True,
                            stop=True,
                        )

            GROUPS = [[0, 1, 2], [3]]
            with nc.named_scope("matmul"):
                for gi, grp in enumerate(GROUPS):
                    for cb in grp:
                        for dt in range(2):
                            ps_t[cb][dt] = psum.tile(
                                [128, 512], fp32,
                                name=f"ps_{cb}_{dt}", tag="ps",
                            )
                    if gi == 0:
                        # warm the PE HAM clock-gate while DMAs stream: junk
                        # matmuls into a bank group A overwrites at t=0
                        with nc.named_scope("warmup"):
                            for w in range(10):
                                nc.tensor.matmul(
                                    out=ps_t[0][0][0:1, :],
                                    lhsT=ones1[:],
                                    rhs=junkw[:],
                                    start=True,
                                    stop=True,
                                )
                    for t in range(KT2):
                        for dt in range(2):
                            for cb in grp:
                                nc.tensor.matmul(
                                    out=ps_t[cb][dt][:],
                                    lhsT=e_t[t][:, :, dt * 128:(dt + 1) * 128],
                                    rhs=a_t[cb][t][:],
                                    start=(t == 0),
                                    stop=(t == KT2 - 1),
                                    perf_mode=DR,
                                )
                    with nc.named_scope(f"reduce{gi}"):
                        for cb in grp:
                            reduce_cb(cb)

            with nc.named_scope("final"):
                red_sb = constp.tile([128, 24], fp32)
                nc.vector.tensor_copy(red_sb[:], red[:, 0:24])
                nc.sync.dma_start(out=out[:], in_=red_sb[:])

    nc.compile()
    return nc


def _get_nc():
    if "nc" not in _CACHE:
        _CACHE["nc"] = _build()
    return _CACHE["nc"]


def kernel(edges, A_s, emb, Ws, bs):
    global LAST_RESULTS
    from concourse.bass_utils import run_bass_kernel_spmd

    e4 = ml_dtypes.float8_e4m3fn
    A = np.asarray(A_s, dtype=np.float32)
    E = np.asarray(emb, dtype=np.float32)
    W = np.asarray(Ws, dtype=np.float32)
    b = np.asarray(bs, dtype=np.float32)
    ed = np.asarray(edges)

    M = W[0].T @ W[1].T @ W[2].T                      # [D, D]
    E2 = E @ M                                        # [N, D]
    b_eff = (b[0] @ W[1].T + b[1]) @ W[2].T + b[2]    # [D]
    s = E2_SCALE_TARGET / np.abs(E2).max()
    E2s = E2 * s
    bias_tot = (0.5 * E2.sum(axis=0) + b_eff) * s     # mean part + bias
    biasb = np.ascontiguousarray(
        np.broadcast_to(
            bias_tot.reshape(2, 128).T[:, :, None].astype(np.float32),
            (128, 2, D),
        )
    )

    # e28[p, t, ko, d] = E2q[t*256 + ko*128 + p, d]
    E2q = np.clip(E2s, -240.0, 240.0).astype(e4)
    e28 = np.ascontiguousarray(
        E2q.reshape(KT2, 2, 128, D).transpose(2, 0, 1, 3)
    )

    # residual of A in fp8 (mean folded into bias above)
    Rq = (A - 0.5).astype(e4)                         # [N, N]

    in_maps = []
    for c in range(N_CORES):
        e = ed[c * EPC:(c + 1) * EPC].astype(np.int64)
        src, dst = e[:, 0], e[:, 1]
        a8 = np.empty((128, 4, KT2, 2, 512), dtype=e4)
        for cb in range(4):
            sl = slice(cb * 256, (cb + 1) * 256)
            nodes = np.concatenate([src[sl], dst[sl]])  # [512]
            Rga = Rq[nodes]                             # [512, 8192] fp8
            # -> [ki, t, ko, col]
            a8[:, cb] = Rga.T.reshape(KT2, 2, 128, 512).transpose(2, 0, 1, 3)
        in_maps.append({"a8": a8, "e28": e28, "biasb": biasb})

    nc = _get_nc()
    kw = {}
    if os.environ.get("KERNEL_TRACE_KW"):
        import json
        kw = json.loads(os.environ["KERNEL_TRACE_KW"])
    res = run_bass_kernel_spmd(nc, in_maps, list(range(N_CORES)), **kw)
    LAST_RESULTS = res

    outs = []
    for c in range(N_CORES):
        r = np.asarray(res.results[c]["out"], dtype=np.float64)  # [128, 24]
        dot = r[:, 0:8].T.reshape(-1)
        ns = r[:, 8:16].T.reshape(-1)
        nd = r[:, 16:24].T.reshape(-1)
        outs.append(np.abs(dot) / np.sqrt(ns * nd))
    out = np.concatenate(outs)
    return np.maximum(out, 0.0).astype(np.float32)


# revision 32
# speedup vs baseline: 1.0150x; 1.0150x over previous
"""Trainium2 Bass kernel for nn_MihGNNEmbeddingTest3 (gnn_message_passing).

Reference math:
    H = mlp(A_s @ emb)          (mlp = 3 linear layers, no activations)
    out[e] = relu(|<H[src_e], H[dst_e]>| / (||H[src_e]|| ||H[dst_e]||))

Since the mlp is affine, fold it:  H = A_s @ (emb @ W_eff^T) + b_eff
(E2 = emb @ W_eff^T precomputed on host).  cos is scale-invariant, so E2
can be globally rescaled to fit fp8 range.

Layout: edge-pre-permuted, collective-free.  Each core computes
H^T columns for exactly the 2048 endpoint nodes of its own 1024 edges
(host gathers the needed A_s rows per core), via fp8-e4m3 DoubleRow
matmuls (K=256 per pass, moving free dim 512).  A_s is shipped as the
residual A-0.5 (quantization error scales with the residual, not the
value; the mean's contribution 0.5*colsum(E2) folds into the bias).
Bias lands via the per-partition bias of the ACT copy that stages
H^T out of PSUM.  dot/||h||^2 reduce over d (the partition dim) with
data-stationary matmuls against a ones column; the final
|dot|*rsqrt(ns*nd) runs on [128, 8] tiles at full lane parallelism.

Columns per core are grouped in 4 blocks of 512 = [src 256 | dst 256]
so each block's dot/norm math reads one PSUM tile; blocks are split in
2 phases of 2 so phase-0 reductions overlap phase-1 matmuls.
"""

import os
import sys

import numpy as np

try:
    import concourse.bass  # noqa: F401
except ImportError:  # pragma: no cover - grading env should have PYTHONPATH set
    for p in ("/opt/trn_rl_repo", "/root/.axon_site/_ro/trn_rl_repo"):
        if os.path.isdir(p) and p not in sys.path:
            sys.path.insert(0, p)

import ml_dtypes

N, D, B = 8192, 256, 8192
N_CORES = 8
EPC = B // N_CORES    # edges per core
COLS = 2 * EPC        # H^T columns per core (src+dst)
KT2 = N // 256        # DoubleRow k-steps (256 contraction each)
JT = EPC // 128       # edge blocks per core
NPH = 2               # phases (2 column-blocks each)
E2_SCALE_TARGET = 200.0

_CACHE = {}
LAST_RESULTS = None  # BassKernelResults of the most recent run (for test.py)


def _build():
    import concourse.bacc as bacc
    import concourse.bass as bass  # noqa: F401
    import concourse.mybir as mybir
    import concourse.tile as tile

    fp32 = mybir.dt.float32
    bf16 = mybir.dt.bfloat16
    fp8 = mybir.dt.float8e4
    DR = mybir.MatmulPerfMode.DoubleRow

    nc = bacc.Bacc(num_devices=N_CORES)
    # a8[p, cb, t, ko, col] = Rq[node(cb*512+col), t*256 + ko*128 + p]
    a8 = nc.declare_dram_parameter(
        "a8", [128, 4, KT2, 2, 512], fp8, isOutput=False
    )
    # e28[p, t, ko, d] = E2q[t*256 + ko*128 + p, d]
    e28 = nc.declare_dram_parameter("e28", [128, KT2, 2, D], fp8, isOutput=False)
    # bias pre-broadcast along the free dim so a single DVE tensor_tensor
    # does PSUM->SBUF staging + bias add (keeps ACT free for the final sqrt)
    biasb = nc.declare_dram_parameter("biasb", [128, 2, D], fp32, isOutput=False)
    # raw dot|ns|nd sums; the final |dot|/sqrt(ns*nd) is host-side
    out = nc.declare_dram_parameter("out", [128, 24], fp32, isOutput=True)

    AB01 = [0, 1, 3, 5, 8, 11, 14, 18, 22, 27, 32]
    AB23 = [0, 2, 4, 8, 12, 16, 20, 24, 28, 32]
    E_BOUNDS = [0, 2, 6, 10, 14, 18, 23, 28, 32]

    with tile.TileContext(nc) as tc:
        with (
            tc.tile_pool(name="ap", bufs=1) as apool,
            tc.tile_pool(name="ep", bufs=1) as epool,
            tc.tile_pool(name="psum", bufs=8, space="PSUM") as psum,
            tc.tile_pool(name="stage", bufs=12) as stage,
            tc.tile_pool(name="const", bufs=1) as constp,
        ):
            a_t = [[None] * KT2 for _ in range(4)]
            e_t = [None] * KT2
            issue_eng = [nc.sync, nc.scalar]
            issue_i = [0]

            def _dma(o, i):
                issue_eng[issue_i[0] % 2].dma_start(out=o, in_=i)
                issue_i[0] += 1

            # bias first (scalar ring so it doesn't delay the first a chunk):
            # a late bias at the back of the DMA queue once stalled the whole
            # reduce (and the 9th PSUM buffer) behind 18MB
            bias_sb = constp.tile([128, 2, D], fp32)
            nc.scalar.dma_start(out=bias_sb[:], in_=biasb[:])

            def load_a(cb, bounds, ci):
                lo, hi = bounds[ci], bounds[ci + 1]
                ac = apool.tile(
                    [128, hi - lo, 2, 512], fp8,
                    name=f"ac_{cb}_{ci}", tag=f"ac{cb}_{ci}",
                )
                _dma(ac[:], a8[:, cb, lo:hi, :, :])
                for t in range(lo, hi):
                    a_t[cb][t] = ac[:, t - lo, :, :]

            def load_e(ci):
                lo, hi = E_BOUNDS[ci], E_BOUNDS[ci + 1]
                ec = epool.tile(
                    [128, hi - lo, 2, D], fp8, name=f"ec_{ci}", tag=f"ec{ci}"
                )
                _dma(ec[:], e28[:, lo:hi, :, :])
                for t in range(lo, hi):
                    e_t[t] = ec[:, t - lo, :, :]

            for ci in range(10):
                if ci < 8:
                    load_e(ci)
                load_a(0, AB01, ci)
                load_a(1, AB01, ci)
                load_a(2, AB01, ci)
            for ci in range(9):
                load_a(3, AB23, ci)

            ones1 = constp.tile([128, 1], bf16)
            nc.vector.memset(ones1[:], 1.0)
            junkw = constp.tile([128, 512], bf16)
            nc.vector.memset(junkw[:], 1.0)
            # preload ACT square+identity tables while DMAs stream (else
            # their 1.3us table loads land on the critical tail)
            junk = constp.tile([128, 1], fp32)
            nc.vector.memset(junk[:], 1.0)
            junk2 = constp.tile([128, 1], fp32)
            nc.scalar.square(junk2[:], junk[:])
            junk4 = constp.tile([128, 1], fp32)
            nc.scalar.activation(
                junk4[:], junk[:],
                mybir.ActivationFunctionType.Identity, bias=0.0,
            )

            # main matmuls + per-block reductions
            red = None  # [128, 24] used: dot j | ns 8+j | nd 16+j
            ps_t = [[None, None] for _ in range(4)]  # [cb][dtile]

            def reduce_cb(cb):
                nonlocal red
                if red is None:
                    # same tag as ps: cycles within the 8-buffer pool (the
                    # 9th allocation reuses a released phase-0 bank)
                    red = psum.tile([128, 512], fp32, name="red", tag="ps")
                tiles = []
                for dt in range(2):
                    ps = ps_t[cb][dt]
                    hs_sb = stage.tile(
                        [128, 256], fp32, name=f"hs_{cb}_{dt}", tag="hs", bufs=8
                    )
                    nc.vector.tensor_tensor(
                        out=hs_sb[:], in0=ps[:, 0:256], in1=bias_sb[:, dt, :],
                        op=mybir.AluOpType.add,
                    )
                    hd_sb = stage.tile(
                        [128, 256], fp32, name=f"hd_{cb}_{dt}", tag="hd", bufs=8
                    )
                    nc.vector.tensor_tensor(
                        out=hd_sb[:], in0=ps[:, 256:512], in1=bias_sb[:, dt, :],
                        op=mybir.AluOpType.add,
                    )
                    P = stage.tile([128, 256], bf16, name=f"P_{cb}_{dt}", tag="P")
                    nc.vector.tensor_tensor(
                        out=P[:], in0=hs_sb[:], in1=hd_sb[:],
                        op=mybir.AluOpType.mult,
                    )
                    S2 = stage.tile([128, 256], bf16, name=f"S2_{cb}_{dt}", tag="S2")
                    nc.scalar.square(S2[:], hs_sb[:])
                    D2 = stage.tile([128, 256], bf16, name=f"D2_{cb}_{dt}", tag="D2")
                    nc.scalar.square(D2[:], hd_sb[:])
                    tiles.append((P, S2, D2))
                # combine the two d-tile partials on DVE so each (chunk,
                # quantity) takes a single reduce matmul (halves PE LDW cost)
                comb = []
                for q in range(3):
                    cq = stage.tile(
                        [128, 256], bf16, name=f"cq_{cb}_{q}", tag=f"cq{q}",
                        bufs=3,
                    )
                    nc.vector.tensor_tensor(
                        out=cq[:], in0=tiles[0][q][:], in1=tiles[1][q][:],
                        op=mybir.AluOpType.add,
                    )
                    comb.append(cq)
                for c in range(2):
                    j = cb * 2 + c
                    for q in range(3):
                        nc.tensor.matmul(
                            out=red[:, 8 * q + j:8 * q + j + 1],
                            lhsT=comb[q][:, c * 128:(c + 1) * 128],
                            rhs=ones1[:],
                            start=True,
                            stop=True,
                        )

            GROUPS = [[0, 1, 2], [3]]
            with nc.named_scope("matmul"):
                for gi, grp in enumerate(GROUPS):
                    for cb in grp:
                        for dt in range(2):
                            ps_t[cb][dt] = psum.tile(
                                [128, 512], fp32,
                                name=f"ps_{cb}_{dt}", tag="ps",
                            )
                    if gi == 0:
                        # warm the PE HAM clock-gate while DMAs stream: junk
                        # matmuls into a bank group A overwrites at t=0
                        with nc.named_scope("warmup"):
                            for w in range(10):
                                nc.tensor.matmul(
                                    out=ps_t[0][0][0:1, :],
                                    lhsT=ones1[:],
                                    rhs=junkw[:],
                                    start=True,
                                    stop=True,
                                )
                    if len(grp) == 1:
                        # dt-outer: the first d-tile finishes at the group's
                        # midpoint so half the reduce staging overlaps the
                        # second half of the matmuls (shrinks the final tail)
                        order = [(t, dt) for dt in range(2)
                                 for t in range(KT2)]
                    else:
                        order = [(t, dt) for t in range(KT2)
                                 for dt in range(2)]
                    for t, dt in order:
                        for cb in grp:
                            nc.tensor.matmul(
                                out=ps_t[cb][dt][:],
                                lhsT=e_t[t][:, :, dt * 128:(dt + 1) * 128],
                                rhs=a_t[cb][t][:],
                                start=(t == 0),
                                stop=(t == KT2 - 1),
                                perf_mode=DR,
                            )
                    with nc.named_scope(f"reduce{gi}"):
                        for cb in grp:
                            reduce_cb(cb)

            with nc.named_scope("final"):
                red_sb = constp.tile([128, 24], fp32)
                nc.vector.tensor_copy(red_sb[:], red[:, 0:24])
                nc.sync.dma_start(out=out[:], in_=red_sb[:])

    nc.compile()
    return nc


def _get_nc():
    if "nc" not in _CACHE:
        _CACHE["nc"] = _build()
    return _CACHE["nc"]


def kernel(edges, A_s, emb, Ws, bs):
    global LAST_RESULTS
    from concourse.bass_utils import run_bass_kernel_spmd

    e4 = ml_dtypes.float8_e4m3fn
    A = np.asarray(A_s, dtype=np.float32)
    E = np.asarray(emb, dtype=np.float32)
    W = np.asarray(Ws, dtype=np.float32)
    b = np.asarray(bs, dtype=np.float32)
    ed = np.asarray(edges)

    M = W[0].T @ W[1].T @ W[2].T                      # [D, D]
    E2 = E @ M                                        # [N, D]
    b_eff = (b[0] @ W[1].T + b[1]) @ W[2].T + b[2]    # [D]
    s = E2_SCALE_TARGET / np.abs(E2).max()
    E2s = E2 * s
    bias_tot = (0.5 * E2.sum(axis=0) + b_eff) * s     # mean part + bias
    biasb = np.ascontiguousarray(
        np.broadcast_to(
            bias_tot.reshape(2, 128).T[:, :, None].astype(np.float32),
            (128, 2, D),
        )
    )

    # e28[p, t, ko, d] = E2q[t*256 + ko*128 + p, d]
    E2q = np.clip(E2s, -240.0, 240.0).astype(e4)
    e28 = np.ascontiguousarray(
        E2q.reshape(KT2, 2, 128, D).transpose(2, 0, 1, 3)
    )

    # residual of A in fp8 (mean folded into bias above)
    Rq = (A - 0.5).astype(e4)                         # [N, N]

    in_maps = []
    for c in range(N_CORES):
        e = ed[c * EPC:(c + 1) * EPC].astype(np.int64)
        src, dst = e[:, 0], e[:, 1]
        a8 = np.empty((128, 4, KT2, 2, 512), dtype=e4)
        for cb in range(4):
            sl = slice(cb * 256, (cb + 1) * 256)
            nodes = np.concatenate([src[sl], dst[sl]])  # [512]
            Rga = Rq[nodes]                             # [512, 8192] fp8
            # -> [ki, t, ko, col]
            a8[:, cb] = Rga.T.reshape(KT2, 2, 128, 512).transpose(2, 0, 1, 3)
        in_maps.append({"a8": a8, "e28": e28, "biasb": biasb})

    nc = _get_nc()
    kw = {}
    if os.environ.get("KERNEL_TRACE_KW"):
        import json
        kw = json.loads(os.environ["KERNEL_TRACE_KW"])
    res = run_bass_kernel_spmd(nc, in_maps, list(range(N_CORES)), **kw)
    LAST_RESULTS = res

    outs = []
    for c in range(N_CORES):
        r = np.asarray(res.results[c]["out"], dtype=np.float64)  # [128, 24]
        dot = r[:, 0:8].T.reshape(-1)
        ns = r[:, 8:16].T.reshape(-1)
        nd = r[:, 16:24].T.reshape(-1)
        outs.append(np.abs(dot) / np.sqrt(ns * nd))
    out = np.concatenate(outs)
    return np.maximum(out, 0.0).astype(np.float32)
